# revision 75
# baseline (speedup 1.0000x reference)
"""Longformer sliding-window self-attention on 8 Trainium2 NeuronCores.

Problem: B=2, S=4096, E=768, H=12 heads, D=64, one-sided window W=256.
Sharding: batch*head parallel - core i handles batch i//4, heads 3*(i%4)..+3.
Each core is fully independent (no collectives).

Design notes (v2, bf16):
  * All matmul operands are bf16 (fp32 PSUM accumulation). fp32r matmuls
    must self-load weights serially (~267ns per MM), which keeps the PE
    array duty cycle below the HAM clock-gate threshold - the whole
    baseline ran at 1.2GHz. bf16 weight loads use FWL (4 elem/cycle) and
    overlap with matmuls, so the PE stays at 2.4GHz.
  * QK projection uses 3 column groups (no head-2 duplication):
    g0=[k0|k1], g1=[q0|q1], g2=[k2|q2] (q pre-scaled by 1/sqrt(D)).
    kT/qT live in kq[128, 3, S] (transposed [d, S] layout).
  * A partition-swapped copy kq2s = [q2|k2] is built via SBUF->SBUF DMA
    so head-2 score matmuls for even blocks run on PE rows 0:63 and odd
    blocks on rows 64:127.
  * Score matmuls have K=64 so they only occupy half the PE rows; they
    are emitted in pairs at partition bases 0/64 (h0 with h1; h2 block m
    with h2 block m+1), which the PE executes concurrently via row-group
    tiling -> 2x throughput.
  * Scores for two adjacent 128-key tiles share one PSUM bank
    ([128, 512] = keys x (2 x 256 queries)), halving Exp instruction
    count on the scalar engine. Band masks are applied multiplicatively
    ({0,1} bf16) AFTER exp, on the otherwise-idle GpSimd engine.
  * v is stored [keys, 65] per (head, key-tile) with a constant ones
    column, so PV matmul row 64 accumulates the softmax denominator.
  * No PE transposes and no division on device: the [65, 512] PV result
    (out^T rows 0:64, denominator row 64) is DMA'd PSUM->DRAM; the host
    does divide + bias + transpose during gather.
  * Projection S-chunks are interleaved with attention block-pairs so
    the scalar engine's exp stream overlaps projection matmul time.
"""

import numpy as np
import ml_dtypes

import concourse.bass as bass
import concourse.bacc as bacc
import concourse.mybir as mybir
import concourse.tile as tile
from concourse.bass_utils import run_bass_kernel_spmd

B, S, E, H, D, W = 2, 4096, 768, 12, 64, 256
NCORES = 8
HPC = 3  # heads per core
QB = 256  # queries per attention block
NBLK = S // QB  # 16
WIN = 3 * QB  # 768-wide key window per block
KI = E // 128  # 6 contraction tiles for projections
PCH = 512  # projection N-chunk (along S)
NCH = S // PCH  # 8
F32 = mybir.dt.float32
BF16 = mybir.dt.bfloat16
Act = mybir.ActivationFunctionType
Alu = mybir.AluOpType
NVG = S // 128  # 32 v key tiles
BF16NP = ml_dtypes.bfloat16


def _masks_np():
    # Multiplicative {0,1} masks, [3, 128, 512]:
    #   [t', p] layout: t' = key offset within 128-tile (partitions),
    #   p = query offset within 256-block; two key tiles side by side.
    # M_A = [ma|mb] applies to pair (kt0, kt1) of a window,
    # M_B = [mc|md] to pair (kt4, kt5). Stored [M_A | M_B | M_A] so the
    # combined masks [M_A|M_B] and [M_B|M_A] are contiguous slices.
    p = np.arange(QB)[None, :]
    t = np.arange(128)[:, None]
    ma = (p <= t)
    mb = (p <= t + 128)
    mc = (p >= t)
    md = (p >= t + 128)
    m_a = np.concatenate([ma, mb], axis=1)
    m_b = np.concatenate([mc, md], axis=1)
    return np.stack([m_a, m_a, m_b, m_b, m_a]).astype(BF16NP)


def _plan_pairs(m):
    """Window-tile pairs for query block m: list of (ktA, ktB, mask_idx).

    Key tile kt covers keys w0+128*kt .. +128, w0 = window start.
    mask_idx indexes _masks_np (None = fully valid)."""
    if m == 0:
        return 0, [(0, 1, None), (2, 3, 1)]
    if m == NBLK - 1:
        return S - WIN, [(2, 3, 0), (4, 5, None)]
    return QB * m - QB, [(0, 1, 0), (2, 3, None), (4, 5, 1)]


def _build_nc():
    nc = bacc.Bacc()
    # weight/mask layouts are pre-arranged on the host so every const DMA
    # is a contiguous read (an on-device rearrange gather costs ~5us)
    ht_d = nc.declare_dram_parameter("hT", [E, S], BF16, isOutput=False)
    wqk_d = nc.declare_dram_parameter("wqk", [128, HPC, KI, 128], BF16, isOutput=False)
    wv_d = nc.declare_dram_parameter("wv", [128, KI, HPC * D], BF16, isOutput=False)
    bqk_d = nc.declare_dram_parameter("bqk", [128, HPC], F32, isOutput=False)
    msk_d = nc.declare_dram_parameter("masks", [128, 5, 2 * QB], BF16, isOutput=False)
    out_d = nc.declare_dram_parameter("out", [HPC, D + 1, S], F32, isOutput=True)

    with tile.TileContext(nc) as tc:
        with (
            tc.tile_pool(name="const", bufs=1) as const,
            tc.tile_pool(name="hpool", bufs=24) as hpool,
            tc.tile_pool(name="work", bufs=32) as work,
            tc.tile_pool(name="obp", bufs=6) as obp,
            tc.tile_pool(name="pp", bufs=2, space="PSUM") as pp,
            tc.tile_pool(name="sp", bufs=2, space="PSUM") as sp,
            tc.tile_pool(name="op", bufs=2, space="PSUM") as op,
        ):
            # ---- persistent tiles (issued from idle engines so the sync
            # queue can start streaming hT immediately) ----
            wqk = const.tile([128, HPC, KI, 128], BF16)
            # split per group: the first projection matmul only needs g=0
            for g in range(HPC):
                nc.scalar.dma_start(wqk[:, g], wqk_d[:, g])
            wv = const.tile([128, KI, HPC * D], BF16)
            # masks laid out [M_A|M_A|M_B|M_B|M_A] so every single and
            # pairwise mask combination is a contiguous slice of one tile
            masks = const.tile([128, 5 * 2 * QB], BF16)
            bqk = const.tile([128, HPC], F32)

            def late_const_dmas():
                # issued after the prologue hT streams: none of these gate
                # the first projection matmuls (bqk first — the chunk-0 kq
                # copies need it soonest)
                nc.scalar.dma_start(bqk, bqk_d[:])
                nc.scalar.dma_start(wv, wv_d[:])
                nc.scalar.dma_start(masks, msk_d.rearrange("p v t -> p (v t)"))

            # transposed [d, S] k/q: g0=[k0|k1], g1=[q0|q1], g2=[k2|q2]
            kq = const.tile([128, HPC, S], BF16)
            # partition-swapped copy of group 2: [q2|k2]
            kq2s = const.tile([128, S], BF16)
            # v in [key-tile, head, keys-in-tile, d+ones] layout
            v_sb = const.tile([128, HPC, NVG, D + 1], BF16)
            nc.gpsimd.memset(v_sb[:], 1.0)  # ones column; data cols overwritten

            def kT(h, g):
                sl = slice(g * 128, (g + 1) * 128)
                return (kq[0:64, 0, sl], kq[64:128, 0, sl])[h]

            def qT(h, b):
                sl = slice(b * QB, (b + 1) * QB)
                return (kq[0:64, 1, sl], kq[64:128, 1, sl])[h]

            # ---- projection chunk: writes kq / kq2s / v_sb for S-range ----
            def proj_dma(c):
                cs = slice(c * PCH, (c + 1) * PCH)
                # prologue chunks split issues across engines (sync alone
                # takes ~4us of serial issue); afterwards the scalar
                # engine is needed at full rate for Exp, so sync only
                hts = []
                for ki in range(KI):
                    t = hpool.tile([128, PCH], BF16, tag="ht")
                    eng = nc.scalar if c < 2 and ki % 2 else nc.sync
                    eng.dma_start(t, ht_d[ki * 128 : (ki + 1) * 128, cs])
                    hts.append(t)
                return hts

            def proj_qk(c, hts):
                cs = slice(c * PCH, (c + 1) * PCH)
                for g in range(HPC):
                    ps = pp.tile([128, PCH], F32, tag="pp")
                    for ki in range(KI):
                        nc.tensor.matmul(
                            ps, wqk[:, g, ki, :], hts[ki],
                            start=(ki == 0), stop=(ki == KI - 1),
                        )
                    # psum -> kq (bf16) with per-partition bias add
                    nc.vector.tensor_scalar(
                        kq[:, g, cs], ps, bqk[:, g : g + 1], None, Alu.add
                    )
                    if g == 2:
                        # build [q2|k2] swap for head-2 score pairing
                        # (issued from gpsimd to keep the sync queue free)
                        nc.gpsimd.dma_start(kq2s[0:64, cs], kq[64:128, 2, cs])
                        nc.gpsimd.dma_start(kq2s[64:128, cs], kq[0:64, 2, cs])

            def proj_v(c, hts):
                for j in range(PCH // 128):
                    g = (PCH // 128) * c + j
                    ps = pp.tile([128, PCH], F32, tag="pp")
                    for ki in range(KI):
                        nc.tensor.matmul(
                            ps[:, 0 : HPC * D],
                            hts[ki][:, j * 128 : (j + 1) * 128],
                            wv[:, ki, :],
                            start=(ki == 0), stop=(ki == KI - 1),
                        )
                    # psum -> v_sb (bf16), de-interleaving the 3 heads
                    nc.vector.tensor_copy(
                        v_sb[:, :, g, 0:D],
                        ps[:, 0 : HPC * D].rearrange("p (h d) -> p h d", h=HPC),
                    )



            # ---- attention over a block pair (m even) ----
            def score_mm(ps_half, lhsT, rhs):
                nc.tensor.matmul(ps_half, lhsT, rhs, start=True, stop=True)

            # masks const layout [M_A|M_A|M_B|M_B|M_A], 512 cols each: every
            # single and pairwise combination is one contiguous slice
            MSK1 = {0: 0, 1: 4 * QB}
            MSK2 = {(0, 0): 0, (1, 1): 4 * QB, (0, 1): 2 * QB, (1, 0): 6 * QB}

            def attn_scores(m):
                pv = {(h, bi): [] for h in range(HPC) for bi in range(2)}
                last = m == NBLK - 2

                def apply_masks(h, et, mL, mR):
                    # mL masks et[:, 0:512], mR masks et[:, 512:1024]
                    eng = nc.gpsimd if h == 2 and not last else nc.vector
                    if mL is not None and mR is not None:
                        off = MSK2[(mL, mR)]
                        eng.tensor_tensor(
                            et[:, 0 : 4 * QB], et[:, 0 : 4 * QB],
                            masks[:, off : off + 4 * QB], Alu.mult,
                        )
                    elif mL is not None:
                        off = MSK1[mL]
                        eng.tensor_tensor(
                            et[:, 0 : 2 * QB], et[:, 0 : 2 * QB],
                            masks[:, off : off + 2 * QB], Alu.mult,
                        )
                    elif mR is not None:
                        off = MSK1[mR]
                        eng.tensor_tensor(
                            et[:, 2 * QB : 4 * QB], et[:, 2 * QB : 4 * QB],
                            masks[:, off : off + 2 * QB], Alu.mult,
                        )

                def stream_of(b, bi):
                    w0, prs = _plan_pairs(b)
                    return [(w0 // 128 + ktA, mi, b, bi) for ktA, _, mi in prs]

                def score_h2(ps, qsl, g, b):
                    # head-2 scores: even blocks on PE rows 0:63 via the
                    # swapped [q2|k2] copy, odd blocks on rows 64:127
                    gs = slice(g * 128, (g + 1) * 128)
                    qs = slice(b * QB, (b + 1) * QB)
                    if b % 2 == 0:
                        score_mm(ps[:, qsl], kq[0:64, 2, gs], kq2s[0:64, qs])
                    else:
                        score_mm(ps[:, qsl], kq2s[64:128, gs], kq[64:128, 2, qs])

                def finish(hh, ps, pair, nq):
                    m0 = pair[0][1]
                    m1 = pair[1][1] if len(pair) > 1 else None
                    et = work.tile([128, 4 * QB], BF16, tag="et")
                    nc.scalar.activation(
                        et[:, 0 : nq * QB], ps[:, 0 : nq * QB], Act.Exp
                    )
                    apply_masks(hh, et, m0, m1)
                    for qi in range(nq):
                        g0, _, b, bi = pair[qi // 2]
                        pv[(hh, bi)].append(
                            (g0 + qi % 2, et[:, qi * QB : (qi + 1) * QB])
                        )

                def emit_groups(h, groups):
                    # groups: list of (g0, mask_idx, block, bi); consecutive
                    # groups share one 2-bank [128, 1024] PSUM tile.
                    for j in range(0, len(groups), 2):
                        pair = groups[j : j + 2]
                        nq = 2 * len(pair)
                        if h < 2:
                            ps0 = sp.tile([128, 4 * QB], F32, tag="sp")
                            ps1 = sp.tile([128, 4 * QB], F32, tag="sp")
                        else:
                            ps2 = sp.tile([128, 4 * QB], F32, tag="sp")
                        for qi in range(nq):
                            g0, _, b, _ = pair[qi // 2]
                            g = g0 + qi % 2
                            qsl = slice(qi * QB, (qi + 1) * QB)
                            if h < 2:
                                score_mm(ps0[:, qsl], kT(0, g), qT(0, b))
                                score_mm(ps1[:, qsl], kT(1, g), qT(1, b))
                            else:
                                score_h2(ps2, qsl, g, b)
                        if h < 2:
                            finish(0, ps0, pair, nq)
                            finish(1, ps1, pair, nq)
                        else:
                            finish(2, ps2, pair, nq)

                # h2 first (its masks queue early on gpsimd); h2's full
                # groups stay within one block (base pairing) and the two
                # blocks' leftover solo groups share one PSUM tile
                sA = stream_of(m, 0)
                sB = stream_of(m + 1, 1)
                emit_groups(2, sA[:2])
                emit_groups(2, sB[:2])
                if len(sA) > 2 or len(sB) > 2:
                    emit_groups(2, sA[2:] + sB[2:])
                # h0/h1 pair at bases 0/64 and can chain across blocks
                emit_groups(0, sA + sB)
                return pv

            def attn_pv(m, pv):
                # PV + denominator (ones column), then DMA transposed result
                for h in (2, 0, 1):
                    po = op.tile([128, 2 * QB], F32, tag="op")
                    for bi in range(2):
                        jobs = pv[(h, bi)]
                        for idx, (g, et_ap) in enumerate(jobs):
                            nc.tensor.matmul(
                                po[0 : D + 1, bi * QB : (bi + 1) * QB],
                                v_sb[:, h, g, :],
                                et_ap,
                                start=(idx == 0), stop=(idx == len(jobs) - 1),
                            )
                    ob = obp.tile([D + 1, 2 * QB], F32, tag="ob")
                    # high priority: this copy frees a PSUM output bank, so it
                    # must not queue behind pending mask ops on the DVE
                    with tc.high_priority():
                        nc.vector.tensor_copy(ob, po[0 : D + 1, :])
                    nc.sync.dma_start(out_d[h, :, m * QB : (m + 2) * QB], ob)

            # ---- schedule: 2 projection chunks ahead of attention; PV of
            # pair m is emitted after the scores of pair m+2 (software
            # pipelining), so the exp->mask chain latency of pair m hides
            # behind pair m+2's independent score matmuls ----
            # V projections are decoupled from QK and slid 4 iterations
            # later: scores need kq two pairs ahead, but v_sb is first read
            # by the deferred PV stage, so proj_v(m/2) at iteration m is
            # always in time — and it gives the late pairs (which have no
            # QK-projection work left) independent PE fill during the
            # exp-latency windows of their score phase.
            hts = {0: proj_dma(0)}
            hts[1] = proj_dma(1)
            late_const_dmas()
            proj_qk(0, hts[0])
            proj_qk(1, hts[1])
            proj_v(0, hts.pop(0))
            pending = None
            for m in range(0, NBLK, 2):
                pv = attn_scores(m)
                c = m // 2 + 2
                if c < NCH:
                    hts[c] = proj_dma(c)
                    proj_qk(c, hts[c])
                cv = m // 2 + 1
                if cv < NCH:
                    proj_v(cv, hts.pop(cv))
                if pending is not None:
                    attn_pv(*pending)
                pending = (m, pv)
            attn_pv(*pending)
    nc.compile()
    return nc


_CACHE = {}


def _get_nc():
    if "nc" not in _CACHE:
        _CACHE["nc"] = _build_nc()
    return _CACHE["nc"]


def make_in_maps(hidden_states, Wq, bq, Wk, bk, Wv, bv):
    hidden_states = np.asarray(hidden_states, dtype=np.float32)
    Wq = np.asarray(Wq, dtype=np.float32)
    Wk = np.asarray(Wk, dtype=np.float32)
    Wv = np.asarray(Wv, dtype=np.float32)
    bq = np.asarray(bq, dtype=np.float32)
    bk = np.asarray(bk, dtype=np.float32)
    scale = 1.0 / float(np.sqrt(D))
    masks = _masks_np()
    in_maps = []
    for core in range(NCORES):
        b = core // (NCORES // B)
        h0 = HPC * (core % (NCORES // B))
        cols = slice(D * h0, D * (h0 + HPC))
        hsl = [slice(D * (h0 + hh), D * (h0 + hh + 1)) for hh in range(HPC)]
        wqk = np.empty((HPC, E, 128), np.float32)
        bqk = np.empty((128, HPC), np.float32)
        # g0=[k0|k1], g1=[q0|q1]*s, g2=[k2|q2*s]
        wqk[0, :, 0:D] = Wk[:, hsl[0]]
        wqk[0, :, D:128] = Wk[:, hsl[1]]
        wqk[1, :, 0:D] = Wq[:, hsl[0]] * scale
        wqk[1, :, D:128] = Wq[:, hsl[1]] * scale
        wqk[2, :, 0:D] = Wk[:, hsl[2]]
        wqk[2, :, D:128] = Wq[:, hsl[2]] * scale
        bqk[0:D, 0] = bk[hsl[0]]
        bqk[D:128, 0] = bk[hsl[1]]
        bqk[0:D, 1] = bq[hsl[0]] * scale
        bqk[D:128, 1] = bq[hsl[1]] * scale
        bqk[0:D, 2] = bk[hsl[2]]
        bqk[D:128, 2] = bq[hsl[2]] * scale
        # device-layout pre-arrangement (contiguous DMAs):
        #   wqk [ki, g, ko, m] from [g, ko*128+ki, m]
        #   wv  [ki, ko, n]    from [ko*128+ki, n]
        #   masks [p, v, t]    from [v, p, t]
        wqk_dev = wqk.reshape(HPC, KI, 128, 128).transpose(2, 0, 1, 3)
        wv_dev = Wv[:, cols].reshape(KI, 128, HPC * D).transpose(1, 0, 2)
        in_maps.append(
            dict(
                hT=np.ascontiguousarray(hidden_states[b].T).astype(BF16NP),
                wqk=np.ascontiguousarray(wqk_dev).astype(BF16NP),
                wv=np.ascontiguousarray(wv_dev).astype(BF16NP),
                bqk=bqk,
                masks=np.ascontiguousarray(masks.transpose(1, 0, 2)),
            )
        )
    return in_maps


def kernel(hidden_states, Wq, bq, Wk, bk, Wv, bv):
    in_maps = make_in_maps(hidden_states, Wq, bq, Wk, bk, Wv, bv)
    res = run_bass_kernel_spmd(_get_nc(), in_maps, list(range(NCORES)))
    kernel.last = res
    bv = np.asarray(bv, dtype=np.float32)
    out = np.empty((B, S, E), np.float32)
    for core in range(NCORES):
        r = res.results[core]["out"]  # [HPC, D+1, S]
        b = core // (NCORES // B)
        h0 = HPC * (core % (NCORES // B))
        for hh in range(HPC):
            cols = slice(D * (h0 + hh), D * (h0 + hh + 1))
            num = r[hh, 0:D, :] / r[hh, D, :][None, :] + bv[cols][:, None]
            out[b, :, cols] = num.T
    return out


# revision 76
# speedup vs baseline: 1.0103x; 1.0103x over previous
"""Longformer sliding-window self-attention on 8 Trainium2 NeuronCores.

Problem: B=2, S=4096, E=768, H=12 heads, D=64, one-sided window W=256.
Sharding: batch*head parallel - core i handles batch i//4, heads 3*(i%4)..+3.
Each core is fully independent (no collectives).

Design notes (v2, bf16):
  * All matmul operands are bf16 (fp32 PSUM accumulation). fp32r matmuls
    must self-load weights serially (~267ns per MM), which keeps the PE
    array duty cycle below the HAM clock-gate threshold - the whole
    baseline ran at 1.2GHz. bf16 weight loads use FWL (4 elem/cycle) and
    overlap with matmuls, so the PE stays at 2.4GHz.
  * QK projection uses 3 column groups (no head-2 duplication):
    g0=[k0|k1], g1=[q0|q1], g2=[k2|q2] (q pre-scaled by 1/sqrt(D)).
    kT/qT live in kq[128, 3, S] (transposed [d, S] layout).
  * A partition-swapped copy kq2s = [q2|k2] is built via SBUF->SBUF DMA
    so head-2 score matmuls for even blocks run on PE rows 0:63 and odd
    blocks on rows 64:127.
  * Score matmuls have K=64 so they only occupy half the PE rows; they
    are emitted in pairs at partition bases 0/64 (h0 with h1; h2 block m
    with h2 block m+1), which the PE executes concurrently via row-group
    tiling -> 2x throughput.
  * Scores for two adjacent 128-key tiles share one PSUM bank
    ([128, 512] = keys x (2 x 256 queries)), halving Exp instruction
    count on the scalar engine. Band masks are applied multiplicatively
    ({0,1} bf16) AFTER exp, on the otherwise-idle GpSimd engine.
  * v is stored [keys, 65] per (head, key-tile) with a constant ones
    column, so PV matmul row 64 accumulates the softmax denominator.
  * No PE transposes and no division on device: the [65, 512] PV result
    (out^T rows 0:64, denominator row 64) is DMA'd PSUM->DRAM; the host
    does divide + bias + transpose during gather.
  * Projection S-chunks are interleaved with attention block-pairs so
    the scalar engine's exp stream overlaps projection matmul time.
"""

import numpy as np
import ml_dtypes

import concourse.bass as bass
import concourse.bacc as bacc
import concourse.mybir as mybir
import concourse.tile as tile
from concourse.bass_utils import run_bass_kernel_spmd

B, S, E, H, D, W = 2, 4096, 768, 12, 64, 256
NCORES = 8
HPC = 3  # heads per core
QB = 256  # queries per attention block
NBLK = S // QB  # 16
WIN = 3 * QB  # 768-wide key window per block
KI = E // 128  # 6 contraction tiles for projections
PCH = 512  # projection N-chunk (along S)
NCH = S // PCH  # 8
F32 = mybir.dt.float32
BF16 = mybir.dt.bfloat16
Act = mybir.ActivationFunctionType
Alu = mybir.AluOpType
NVG = S // 128  # 32 v key tiles
BF16NP = ml_dtypes.bfloat16


def _masks_np():
    # Multiplicative {0,1} masks, [3, 128, 512]:
    #   [t', p] layout: t' = key offset within 128-tile (partitions),
    #   p = query offset within 256-block; two key tiles side by side.
    # M_A = [ma|mb] applies to pair (kt0, kt1) of a window,
    # M_B = [mc|md] to pair (kt4, kt5). Stored [M_A | M_B | M_A] so the
    # combined masks [M_A|M_B] and [M_B|M_A] are contiguous slices.
    p = np.arange(QB)[None, :]
    t = np.arange(128)[:, None]
    ma = (p <= t)
    mb = (p <= t + 128)
    mc = (p >= t)
    md = (p >= t + 128)
    m_a = np.concatenate([ma, mb], axis=1)
    m_b = np.concatenate([mc, md], axis=1)
    return np.stack([m_a, m_a, m_b, m_b, m_a]).astype(BF16NP)


def _plan_pairs(m):
    """Window-tile pairs for query block m: list of (ktA, ktB, mask_idx).

    Key tile kt covers keys w0+128*kt .. +128, w0 = window start.
    mask_idx indexes _masks_np (None = fully valid)."""
    if m == 0:
        return 0, [(0, 1, None), (2, 3, 1)]
    if m == NBLK - 1:
        return S - WIN, [(2, 3, 0), (4, 5, None)]
    return QB * m - QB, [(0, 1, 0), (2, 3, None), (4, 5, 1)]


def _build_nc():
    nc = bacc.Bacc()
    # weight/mask layouts are pre-arranged on the host so every const DMA
    # is a contiguous read (an on-device rearrange gather costs ~5us)
    ht_d = nc.declare_dram_parameter("hT", [E, S], BF16, isOutput=False)
    wqk_d = nc.declare_dram_parameter("wqk", [128, HPC, KI, 128], BF16, isOutput=False)
    wv_d = nc.declare_dram_parameter("wv", [128, KI, HPC * D], BF16, isOutput=False)
    bqk_d = nc.declare_dram_parameter("bqk", [128, HPC], F32, isOutput=False)
    msk_d = nc.declare_dram_parameter("masks", [128, 5, 2 * QB], BF16, isOutput=False)
    out_d = nc.declare_dram_parameter("out", [HPC, D + 1, S], F32, isOutput=True)

    with tile.TileContext(nc) as tc:
        with (
            tc.tile_pool(name="const", bufs=1) as const,
            tc.tile_pool(name="hpool", bufs=24) as hpool,
            tc.tile_pool(name="work", bufs=32) as work,
            tc.tile_pool(name="obp", bufs=6) as obp,
            tc.tile_pool(name="pp", bufs=2, space="PSUM") as pp,
            tc.tile_pool(name="sp", bufs=2, space="PSUM") as sp,
            tc.tile_pool(name="op", bufs=2, space="PSUM") as op,
        ):
            # ---- persistent tiles (issued from idle engines so the sync
            # queue can start streaming hT immediately) ----
            wqk = const.tile([128, HPC, KI, 128], BF16)
            # split per group: the first projection matmul only needs g=0
            for g in range(HPC):
                nc.scalar.dma_start(wqk[:, g], wqk_d[:, g])
            wv = const.tile([128, KI, HPC * D], BF16)
            # masks laid out [M_A|M_A|M_B|M_B|M_A] so every single and
            # pairwise mask combination is a contiguous slice of one tile
            masks = const.tile([128, 5 * 2 * QB], BF16)
            bqk = const.tile([128, HPC], F32)

            def late_const_dmas():
                # issued after the prologue hT streams: none of these gate
                # the first projection matmuls (bqk first — the chunk-0 kq
                # copies need it soonest)
                nc.scalar.dma_start(bqk, bqk_d[:])
                nc.scalar.dma_start(wv, wv_d[:])
                nc.scalar.dma_start(masks, msk_d.rearrange("p v t -> p (v t)"))

            # transposed [d, S] k/q: g0=[k0|k1], g1=[q0|q1], g2=[k2|q2]
            kq = const.tile([128, HPC, S], BF16)
            # partition-swapped copy of group 2: [q2|k2]
            kq2s = const.tile([128, S], BF16)
            # v in [key-tile, head, keys-in-tile, d+ones] layout
            v_sb = const.tile([128, HPC, NVG, D + 1], BF16)
            nc.gpsimd.memset(v_sb[:], 1.0)  # ones column; data cols overwritten

            def kT(h, g):
                sl = slice(g * 128, (g + 1) * 128)
                return (kq[0:64, 0, sl], kq[64:128, 0, sl])[h]

            def qT(h, b):
                sl = slice(b * QB, (b + 1) * QB)
                return (kq[0:64, 1, sl], kq[64:128, 1, sl])[h]

            # ---- projection chunk: writes kq / kq2s / v_sb for S-range ----
            def proj_dma(c):
                cs = slice(c * PCH, (c + 1) * PCH)
                # prologue chunks split issues across engines (sync alone
                # takes ~4us of serial issue); afterwards the scalar
                # engine is needed at full rate for Exp, so sync only
                hts = []
                for ki in range(KI):
                    t = hpool.tile([128, PCH], BF16, tag="ht")
                    eng = nc.scalar if c < 2 and ki % 2 else nc.sync
                    eng.dma_start(t, ht_d[ki * 128 : (ki + 1) * 128, cs])
                    hts.append(t)
                return hts

            def proj_qk(c, hts):
                cs = slice(c * PCH, (c + 1) * PCH)
                for g in range(HPC):
                    ps = pp.tile([128, PCH], F32, tag="pp")
                    for ki in range(KI):
                        nc.tensor.matmul(
                            ps, wqk[:, g, ki, :], hts[ki],
                            start=(ki == 0), stop=(ki == KI - 1),
                        )
                    # psum -> kq (bf16) with per-partition bias add
                    nc.vector.tensor_scalar(
                        kq[:, g, cs], ps, bqk[:, g : g + 1], None, Alu.add
                    )
                    if g == 2:
                        # build [q2|k2] swap for head-2 score pairing
                        # (issued from gpsimd to keep the sync queue free)
                        nc.gpsimd.dma_start(kq2s[0:64, cs], kq[64:128, 2, cs])
                        nc.gpsimd.dma_start(kq2s[64:128, cs], kq[0:64, 2, cs])

            def proj_v(c, hts):
                for j in range(PCH // 128):
                    g = (PCH // 128) * c + j
                    ps = pp.tile([128, PCH], F32, tag="pp")
                    for ki in range(KI):
                        nc.tensor.matmul(
                            ps[:, 0 : HPC * D],
                            hts[ki][:, j * 128 : (j + 1) * 128],
                            wv[:, ki, :],
                            start=(ki == 0), stop=(ki == KI - 1),
                        )
                    # psum -> v_sb (bf16), de-interleaving the 3 heads
                    nc.vector.tensor_copy(
                        v_sb[:, :, g, 0:D],
                        ps[:, 0 : HPC * D].rearrange("p (h d) -> p h d", h=HPC),
                    )



            # ---- attention over a block pair (m even) ----
            def score_mm(ps_half, lhsT, rhs):
                nc.tensor.matmul(ps_half, lhsT, rhs, start=True, stop=True)

            # masks const layout [M_A|M_A|M_B|M_B|M_A], 512 cols each: every
            # single and pairwise combination is one contiguous slice
            MSK1 = {0: 0, 1: 4 * QB}
            MSK2 = {(0, 0): 0, (1, 1): 4 * QB, (0, 1): 2 * QB, (1, 0): 6 * QB}

            def attn_scores(m):
                pv = {(h, bi): [] for h in range(HPC) for bi in range(2)}
                last = m == NBLK - 2

                def apply_masks(h, et, mL, mR):
                    # mL masks et[:, 0:512], mR masks et[:, 512:1024]
                    eng = nc.gpsimd if h == 2 and not last else nc.vector
                    if mL is not None and mR is not None:
                        off = MSK2[(mL, mR)]
                        eng.tensor_tensor(
                            et[:, 0 : 4 * QB], et[:, 0 : 4 * QB],
                            masks[:, off : off + 4 * QB], Alu.mult,
                        )
                    elif mL is not None:
                        off = MSK1[mL]
                        eng.tensor_tensor(
                            et[:, 0 : 2 * QB], et[:, 0 : 2 * QB],
                            masks[:, off : off + 2 * QB], Alu.mult,
                        )
                    elif mR is not None:
                        off = MSK1[mR]
                        eng.tensor_tensor(
                            et[:, 2 * QB : 4 * QB], et[:, 2 * QB : 4 * QB],
                            masks[:, off : off + 2 * QB], Alu.mult,
                        )

                def stream_of(b, bi):
                    w0, prs = _plan_pairs(b)
                    return [(w0 // 128 + ktA, mi, b, bi) for ktA, _, mi in prs]

                def score_h2(ps, qsl, g, b):
                    # head-2 scores: even blocks on PE rows 0:63 via the
                    # swapped [q2|k2] copy, odd blocks on rows 64:127
                    gs = slice(g * 128, (g + 1) * 128)
                    qs = slice(b * QB, (b + 1) * QB)
                    if b % 2 == 0:
                        score_mm(ps[:, qsl], kq[0:64, 2, gs], kq2s[0:64, qs])
                    else:
                        score_mm(ps[:, qsl], kq2s[64:128, gs], kq[64:128, 2, qs])

                def finish(hh, ps, pair, nq):
                    m0 = pair[0][1]
                    m1 = pair[1][1] if len(pair) > 1 else None
                    et = work.tile([128, 4 * QB], BF16, tag="et")
                    nc.scalar.activation(
                        et[:, 0 : nq * QB], ps[:, 0 : nq * QB], Act.Exp
                    )
                    apply_masks(hh, et, m0, m1)
                    for qi in range(nq):
                        g0, _, b, bi = pair[qi // 2]
                        pv[(hh, bi)].append(
                            (g0 + qi % 2, et[:, qi * QB : (qi + 1) * QB])
                        )

                def emit_groups(h, groups):
                    # groups: list of (g0, mask_idx, block, bi); consecutive
                    # groups share one 2-bank [128, 1024] PSUM tile.
                    for j in range(0, len(groups), 2):
                        pair = groups[j : j + 2]
                        nq = 2 * len(pair)
                        if h < 2:
                            ps0 = sp.tile([128, 4 * QB], F32, tag="sp")
                            ps1 = sp.tile([128, 4 * QB], F32, tag="sp")
                        else:
                            ps2 = sp.tile([128, 4 * QB], F32, tag="sp")
                        for qi in range(nq):
                            g0, _, b, _ = pair[qi // 2]
                            g = g0 + qi % 2
                            qsl = slice(qi * QB, (qi + 1) * QB)
                            if h < 2:
                                score_mm(ps0[:, qsl], kT(0, g), qT(0, b))
                                score_mm(ps1[:, qsl], kT(1, g), qT(1, b))
                            else:
                                score_h2(ps2, qsl, g, b)
                        if h < 2:
                            finish(0, ps0, pair, nq)
                            finish(1, ps1, pair, nq)
                        else:
                            finish(2, ps2, pair, nq)

                # h0/h1 first: their DVE-masked et chains drain fastest, so
                # they don't hold the PSUM rotation behind h2's slower
                # gpsimd-masked chains (whose PV consumers are deferred a
                # full pair anyway). h2's full groups stay within one block
                # (base pairing); the two blocks' leftover solo groups
                # share one PSUM tile.
                sA = stream_of(m, 0)
                sB = stream_of(m + 1, 1)
                emit_groups(0, sA + sB)
                emit_groups(2, sA[:2])
                emit_groups(2, sB[:2])
                if len(sA) > 2 or len(sB) > 2:
                    emit_groups(2, sA[2:] + sB[2:])
                return pv

            def attn_pv(m, pv):
                # PV + denominator (ones column), then DMA transposed result
                for h in (2, 0, 1):
                    po = op.tile([128, 2 * QB], F32, tag="op")
                    for bi in range(2):
                        jobs = pv[(h, bi)]
                        for idx, (g, et_ap) in enumerate(jobs):
                            nc.tensor.matmul(
                                po[0 : D + 1, bi * QB : (bi + 1) * QB],
                                v_sb[:, h, g, :],
                                et_ap,
                                start=(idx == 0), stop=(idx == len(jobs) - 1),
                            )
                    ob = obp.tile([D + 1, 2 * QB], F32, tag="ob")
                    # high priority: this copy frees a PSUM output bank, so it
                    # must not queue behind pending mask ops on the DVE
                    with tc.high_priority():
                        nc.vector.tensor_copy(ob, po[0 : D + 1, :])
                    nc.sync.dma_start(out_d[h, :, m * QB : (m + 2) * QB], ob)

            # ---- schedule: 2 projection chunks ahead of attention; PV of
            # pair m is emitted after the scores of pair m+2 (software
            # pipelining), so the exp->mask chain latency of pair m hides
            # behind pair m+2's independent score matmuls ----
            # V projections are decoupled from QK and slid 4 iterations
            # later: scores need kq two pairs ahead, but v_sb is first read
            # by the deferred PV stage, so proj_v(m/2) at iteration m is
            # always in time — and it gives the late pairs (which have no
            # QK-projection work left) independent PE fill during the
            # exp-latency windows of their score phase.
            hts = {0: proj_dma(0)}
            hts[1] = proj_dma(1)
            late_const_dmas()
            proj_qk(0, hts[0])
            proj_qk(1, hts[1])
            proj_v(0, hts.pop(0))
            pending = None
            for m in range(0, NBLK, 2):
                pv = attn_scores(m)
                c = m // 2 + 2
                if c < NCH:
                    hts[c] = proj_dma(c)
                    proj_qk(c, hts[c])
                cv = m // 2 + 1
                if cv < NCH:
                    proj_v(cv, hts.pop(cv))
                if pending is not None:
                    attn_pv(*pending)
                pending = (m, pv)
            attn_pv(*pending)
    nc.compile()
    return nc


_CACHE = {}


def _get_nc():
    if "nc" not in _CACHE:
        _CACHE["nc"] = _build_nc()
    return _CACHE["nc"]


def make_in_maps(hidden_states, Wq, bq, Wk, bk, Wv, bv):
    hidden_states = np.asarray(hidden_states, dtype=np.float32)
    Wq = np.asarray(Wq, dtype=np.float32)
    Wk = np.asarray(Wk, dtype=np.float32)
    Wv = np.asarray(Wv, dtype=np.float32)
    bq = np.asarray(bq, dtype=np.float32)
    bk = np.asarray(bk, dtype=np.float32)
    scale = 1.0 / float(np.sqrt(D))
    masks = _masks_np()
    in_maps = []
    for core in range(NCORES):
        b = core // (NCORES // B)
        h0 = HPC * (core % (NCORES // B))
        cols = slice(D * h0, D * (h0 + HPC))
        hsl = [slice(D * (h0 + hh), D * (h0 + hh + 1)) for hh in range(HPC)]
        wqk = np.empty((HPC, E, 128), np.float32)
        bqk = np.empty((128, HPC), np.float32)
        # g0=[k0|k1], g1=[q0|q1]*s, g2=[k2|q2*s]
        wqk[0, :, 0:D] = Wk[:, hsl[0]]
        wqk[0, :, D:128] = Wk[:, hsl[1]]
        wqk[1, :, 0:D] = Wq[:, hsl[0]] * scale
        wqk[1, :, D:128] = Wq[:, hsl[1]] * scale
        wqk[2, :, 0:D] = Wk[:, hsl[2]]
        wqk[2, :, D:128] = Wq[:, hsl[2]] * scale
        bqk[0:D, 0] = bk[hsl[0]]
        bqk[D:128, 0] = bk[hsl[1]]
        bqk[0:D, 1] = bq[hsl[0]] * scale
        bqk[D:128, 1] = bq[hsl[1]] * scale
        bqk[0:D, 2] = bk[hsl[2]]
        bqk[D:128, 2] = bq[hsl[2]] * scale
        # device-layout pre-arrangement (contiguous DMAs):
        #   wqk [ki, g, ko, m] from [g, ko*128+ki, m]
        #   wv  [ki, ko, n]    from [ko*128+ki, n]
        #   masks [p, v, t]    from [v, p, t]
        wqk_dev = wqk.reshape(HPC, KI, 128, 128).transpose(2, 0, 1, 3)
        wv_dev = Wv[:, cols].reshape(KI, 128, HPC * D).transpose(1, 0, 2)
        in_maps.append(
            dict(
                hT=np.ascontiguousarray(hidden_states[b].T).astype(BF16NP),
                wqk=np.ascontiguousarray(wqk_dev).astype(BF16NP),
                wv=np.ascontiguousarray(wv_dev).astype(BF16NP),
                bqk=bqk,
                masks=np.ascontiguousarray(masks.transpose(1, 0, 2)),
            )
        )
    return in_maps


def kernel(hidden_states, Wq, bq, Wk, bk, Wv, bv):
    in_maps = make_in_maps(hidden_states, Wq, bq, Wk, bk, Wv, bv)
    res = run_bass_kernel_spmd(_get_nc(), in_maps, list(range(NCORES)))
    kernel.last = res
    bv = np.asarray(bv, dtype=np.float32)
    out = np.empty((B, S, E), np.float32)
    for core in range(NCORES):
        r = res.results[core]["out"]  # [HPC, D+1, S]
        b = core // (NCORES // B)
        h0 = HPC * (core % (NCORES // B))
        for hh in range(HPC):
            cols = slice(D * (h0 + hh), D * (h0 + hh + 1))
            num = r[hh, 0:D, :] / r[hh, D, :][None, :] + bv[cols][:, None]
            out[b, :, cols] = num.T
    return out
